# revision 33
# baseline (speedup 1.0000x reference)
"""Trainium2 Bass kernel for nn_Actor (gnn_message_passing).

Math (per batch b):
  k_mu = kv[..., :128], v_mu = kv[..., 128:256]
  rel[n,m]  = <k_mu[n], v_mu[m]> / sqrt(128)
  P[n,m,:]  = pos[n] - pos[m];  Pn = P / (||P|| + eps)
  out[n,:]  = 0.01 * tanh( sum_m Pn[n,m,:] * rel[n,m] )

Factored form used here (avoids materializing [N,N,3]):
  W[n,m]   = rel[n,m] / ||P[n,m]||
  out[n,d] = 0.01 * tanh( pos16[n,d] * s[n] - (W @ [pos16|1])[n,d] ),
  s[n] = sum_m W[n,m].  The diagonal W[n,n] cancels exactly between the
  two terms because the same fp16 positions are used on both sides.

Per-core pipeline (2 batches data-parallel over B=16 across 8 cores),
one batch at a time, per m-tile t (W^T layout [m, n]):
  PE:      d2_ps[128,1024]  = K=7 aug matmul  (pos16, |p|^2 split hi/lo)
  ScalarE: ninv[128,1024]   = ACT Abs_reciprocal_sqrt(E * d2)  -> fp16
           (folds the 1/sqrt(E) into the activation's free scale;
            |.| launders tiny-negative d2 from fp32 PSUM rounding)
  PE:      rel_ps[128,512]  = K=128 matmul halves (fp16 operands)
  VectorE: W[:,half]        = rel * min(ninv, CAP)   (custom DVE op;
           the cap kills the rsqrt(~0)=inf on the diagonal)
  PE:      P[4,512]        += X_t^T @ W-half, the two n-halves packed
           into one PSUM bank at partitions 0-3 / 32-35 (col groups)
Epilogue per batch: drain P, PE-transpose to n-major, pre = pos16*s - A;
single deferred tanh for both batches, scale, DMA out.
"""

import time

import numpy as np

import concourse.bass as bass
import concourse.bacc as bacc
import concourse.mybir as mybir
import concourse.tile as tile
import concourse.dve_ops as dve_ops
from concourse.bass_utils import run_bass_kernel_spmd
from concourse.dve_spec import Spec, minn
from concourse.dve_uop import DveOpSpec
from concourse.masks import make_identity

F32 = mybir.dt.float32
F16 = mybir.dt.float16
AF = mybir.ActivationFunctionType

B, N, CKV = 16, 1024, 259
E = 128
NCORES = 8
BPC = B // NCORES          # batches per core
NT = N // 128              # 128-row m-tiles per batch
KA = 7                     # augmented contraction size for the d2 matmul
ACTION_SCALE = 0.01
CAP = float(np.float16(1.0 / np.sqrt(E * 5e-6)))   # ninv cap (d2 floor 5e-6)


def _register_capmul():
    name = "CAPMUL_GNN"
    if name in dve_ops._SUB_OPCODE_FOR_NAME:
        return next(op for op in dve_ops.OPS if op.name == name)
    from concourse.dve_spec import Src0, Src1, C0, lower

    body = Src0 * minn(Src1, C0)

    def _ref(in0, in1, s0, s1, imm2):
        in0 = np.asarray(in0, np.float32)
        in1 = np.asarray(in1, np.float32)
        return (in0 * np.minimum(in1, np.float32(s0))).astype(np.float32)

    spec = Spec(body=body, reference=_ref)
    opcode = dve_ops._CUSTOM_DVE_ROW_BASE + len(dve_ops.OPS)
    shas = {}
    for ver in ("v3", "v4"):
        try:
            uops = lower(spec, ver=ver)
            shas[ver] = DveOpSpec(
                name=name, opcode=opcode, uops=uops, rd1_en=True
            ).sha(ver)
        except Exception:
            pass
    op = dve_ops.DveOp(name, spec, subdim=False, uops_sha=shas)
    dve_ops.OPS.append(op)
    dve_ops.CUSTOM_DVE_SPECS[name] = spec
    dve_ops._SUB_OPCODE_FOR_NAME[name] = opcode
    return op


CAPMUL_GNN = _register_capmul()


def build_nc():
    nc = bacc.Bacc("TRN2", target_bir_lowering=False, debug=False)
    kv_ext = nc.declare_dram_parameter("kv", [BPC, N, CKV], F32, isOutput=False)
    pos_ext = nc.declare_dram_parameter("positions", [BPC, N, 3], F32, isOutput=False)
    # output in SBUF-native layout [p, b, t, d] (row n = 8p + t); host rearranges
    out_ext = nc.declare_dram_parameter("out", [128, BPC, NT, 3], F32, isOutput=True)

    with tile.TileContext(nc) as tc:
        with (
            tc.tile_pool(name="const", bufs=1) as constp,
            tc.tile_pool(name="kv16", bufs=2) as kv16p,
            tc.tile_pool(name="kvT", bufs=2) as kvTp,
            tc.tile_pool(name="aug", bufs=2) as augp,
            tc.tile_pool(name="ninv", bufs=3) as ninvp,
            tc.tile_pool(name="wt", bufs=4) as wtp,
            tc.tile_pool(name="epi", bufs=2) as epip,
            # PSUM: pswork 2x(2 banks) + rel 3x(1 bank) + P 1 bank = 8 banks
            tc.tile_pool(name="pswork", bufs=2, space="PSUM") as pswork,
            tc.tile_pool(name="psrel", bufs=2, space="PSUM") as psrel,
            tc.tile_pool(name="psP", bufs=1, space="PSUM") as psP,
        ):
            # ---- PE warm-up primer: dependency-free back-to-back matmuls
            # trip the HAM clock gate to 2.4 GHz during the DMA prologue
            warm_in = constp.tile([128, 512], F16)
            nc.vector.memset(warm_in[:, :], 0.0)
            warm_ps = psrel.tile([128, 512], F32, tag="rel")
            for i in range(14):
                nc.tensor.matmul(
                    warm_ps[:, :],
                    lhsT=warm_in[:, 0:128],
                    rhs=warm_in[:, :],
                    start=(i == 0),
                    stop=(i == 13),
                )
            warm_sink = constp.tile([128, 1], F32)
            nc.vector.tensor_copy(warm_sink[:, :], warm_ps[:, 0:1])

            identity16 = constp.tile([128, 128], F16)
            make_identity(nc, identity16[:, :])
            identity32 = constp.tile([128, 128], F32)
            make_identity(nc, identity32[:, :])

            # pre-tanh values for both batches; single deferred tanh keeps
            # one ACT-table switch for the whole kernel
            pre_all = constp.tile([128, BPC, NT, 3], F32)

            kv16s, posf = {}, {}
            for b in range(BPC):
                posf[b] = augp.tile([128, NT, 3], F32, tag="posf", name=f"posf{b}")
                nc.scalar.dma_start(
                    out=posf[b][:, :, :],
                    in_=pos_ext[b].rearrange("(p t) d -> p t d", p=128),
                )
            for b in range(BPC):
                # split each batch's kv across two DMA queues / two tiles so
                # transposes can start as soon as the first half lands
                kv16s[b] = [
                    kv16p.tile([128, NT // 2, 2 * E], F16, tag=f"kv{h}",
                               name=f"kv16_{b}{h}")
                    for h in range(2)
                ]
                src = kv_ext[b].rearrange("(p t) c -> p t c", p=128)
                nc.gpsimd.dma_start(
                    out=kv16s[b][0][:, :, :],
                    in_=src[:, 0 : NT // 2, 0 : 2 * E],
                )
                nc.gpsimd.dma_start(
                    out=kv16s[b][1][:, :, :],
                    in_=src[:, NT // 2 : NT, 0 : 2 * E],
                )

            kmuT, vmuT, augTL, augTR, X, pos16f = {}, {}, {}, {}, {}, {}

            def emit_kvT(b):
                # ---- transpose k/v to [e, n] fp16 via PE, per DMA chunk ----
                kT_ps = pswork.tile([128, N], F16, tag="work")
                vT_ps = pswork.tile([128, N], F16, tag="work")
                for t in range(NT):
                    kv16 = kv16s[b][t // (NT // 2)]
                    tt = t % (NT // 2)
                    nc.tensor.transpose(
                        kT_ps[:, 128 * t : 128 * (t + 1)],
                        kv16[:, tt, 0:E],
                        identity16[:, :],
                    )
                    nc.tensor.transpose(
                        vT_ps[:, 128 * t : 128 * (t + 1)],
                        kv16[:, tt, E : 2 * E],
                        identity16[:, :],
                    )
                kmuT[b] = kvTp.tile([128, N], F16, tag="kmuT", name=f"kmuT{b}")
                nc.vector.tensor_copy(kmuT[b][:, :], kT_ps[:, :])
                vmuT[b] = kvTp.tile([128, N], F16, tag="vmuT", name=f"vmuT{b}")
                nc.scalar.copy(vmuT[b][:, :], vT_ps[:, :])

            def emit_aug(b):
                # ---- positions: fp16 round, |p|^2 split hi/lo ----
                pf = posf[b]
                pos16 = augp.tile([128, NT, 3], F16, tag="p16")
                nc.gpsimd.tensor_copy(pos16[:, :, :], pf[:, :, :])
                pos16f[b] = augp.tile(
                    [128, NT, 3], F32, tag="p16f", name=f"p16f{b}"
                )
                nc.gpsimd.tensor_copy(pos16f[b][:, :, :], pos16[:, :, :])
                sq3 = augp.tile([128, NT, 3], F32, tag="sq3")
                nc.gpsimd.tensor_mul(sq3[:, :, :], pos16f[b][:, :, :], pos16f[b][:, :, :])
                pn2 = augp.tile([128, NT, 1], F32, tag="pn2")
                nc.vector.tensor_reduce(
                    out=pn2[:, :, :],
                    in_=sq3[:, :, :],
                    op=mybir.AluOpType.add,
                    axis=mybir.AxisListType.X,
                )
                h16 = augp.tile([128, NT, 1], F16, tag="h16")
                nc.gpsimd.tensor_copy(h16[:, :, :], pn2[:, :, :])
                h32 = augp.tile([128, NT, 1], F32, tag="h32")
                nc.gpsimd.tensor_copy(h32[:, :, :], h16[:, :, :])
                l32 = augp.tile([128, NT, 1], F32, tag="l32")
                nc.gpsimd.tensor_sub(l32[:, :, :], pn2[:, :, :], h32[:, :, :])

                # ---- augmented block [L(7) | R(7)] in n-major, one PE-T ----
                # L rows (lhsT): [p(3), 1, 1, h, l]; R rows (rhs): [-2p(3), h, l, 1, 1]
                # The K=14 matmul of [L;R] against the row-swapped [R;L] buffer
                # computes 2*d2; the 1/2 folds into the ACT scale.
                augb = augp.tile([128, NT, 2 * KA], F16, tag="augb")
                nc.gpsimd.tensor_copy(augb[:, :, 0:3], pos16[:, :, :])
                nc.gpsimd.memset(augb[:, :, 3:5], 1.0)
                nc.gpsimd.tensor_copy(augb[:, :, 5:6], h16[:, :, :])
                nc.gpsimd.tensor_copy(augb[:, :, 6:7], l32[:, :, :])
                nc.gpsimd.tensor_scalar_mul(augb[:, :, 7:10], pos16[:, :, :], -2.0)
                nc.gpsimd.tensor_copy(augb[:, :, 10:11], h16[:, :, :])
                nc.gpsimd.tensor_copy(augb[:, :, 11:12], l32[:, :, :])
                nc.gpsimd.memset(augb[:, :, 12:14], 1.0)

                aT_ps = pswork.tile([2 * KA, N], F16, tag="work")
                for t in range(NT):
                    nc.tensor.transpose(
                        aT_ps[:, 128 * t : 128 * (t + 1)],
                        augb[:, t, :],
                        identity16[:, :],
                    )
                augTL[b] = augp.tile([2 * KA, N], F16, tag="augTL", name=f"augTL{b}")
                nc.vector.tensor_copy(augTL[b][:, :], aT_ps[:, :])
                # row-swapped copy [R;L] via SBUF->SBUF DMA (engines can't
                # cross partitions; DMA can)
                augTR[b] = augp.tile([2 * KA, N], F16, tag="augTR", name=f"augTR{b}")
                nc.sync.dma_start(
                    out=augTR[b][0:KA, :], in_=augTL[b][KA : 2 * KA, :]
                )
                nc.sync.dma_start(
                    out=augTR[b][KA : 2 * KA, :], in_=augTL[b][0:KA, :]
                )

                # ---- X = [pos16 | 1] for the P accumulation ----
                X[b] = augp.tile([128, NT, 4], F16, tag="X", name=f"X{b}")
                nc.gpsimd.tensor_copy(X[b][:, :, 0:3], pos16[:, :, :])
                nc.gpsimd.memset(X[b][:, :, 3:4], 1.0)

            def emit_warm_mm():
                # dependency-free filler matmul into unused partitions of the
                # P bank: keeps the PE HAM activity monitor at K=8/8 (2.4 GHz)
                # through pipeline bubbles; output is garbage, never read
                nc.tensor.matmul(
                    P_all[1][64:68, :],
                    lhsT=warm_in[:, 0:4],
                    rhs=warm_in[:, :],
                    start=True,
                    stop=True,
                    tile_position=(0, 64),
                    skip_group_check=True,
                )

            def emit_tile(b, t, P_ps):
                d2_ps = pswork.tile([128, N], F32, tag="work")
                for h in range(2):
                    cs = slice(512 * h, 512 * (h + 1))
                    nc.tensor.matmul(
                        d2_ps[:, cs],
                        lhsT=augTL[b][:, 128 * t : 128 * (t + 1)],
                        rhs=augTR[b][:, cs],
                        start=True,
                        stop=True,
                    )
                ninv = ninvp.tile([128, N], F16)
                nc.scalar.activation(
                    ninv[:, :], d2_ps[:, :], AF.Abs_reciprocal_sqrt,
                    scale=float(E) / 2.0,
                )
                emit_warm_mm()
                w16 = wtp.tile([128, N], F16)
                for h in range(2):
                    cs = slice(512 * h, 512 * (h + 1))
                    rel_ps = psrel.tile([128, 512], F32, tag="rel")
                    nc.tensor.matmul(
                        rel_ps[:, :],
                        lhsT=vmuT[b][:, 128 * t : 128 * (t + 1)],
                        rhs=kmuT[b][:, cs],
                        start=True,
                        stop=True,
                    )
                    nc.vector._custom_dve(
                        CAPMUL_GNN,
                        out=w16[:, cs],
                        in0=rel_ps[:, :],
                        in1=ninv[:, cs],
                        s0=CAP,
                        s1=0.0,
                    )
                    # P accumulation: n-half h -> partitions 32h..32h+4 of the
                    # batch's P bank (PE column group h, halves run concurrent)
                    pb = 32 * h
                    nc.tensor.matmul(
                        P_ps[pb : pb + 4, :],
                        lhsT=X[b][:, t, :],
                        rhs=w16[:, cs],
                        start=(t == 0),
                        stop=(t == NT - 1),
                        tile_position=(0, pb),
                        skip_group_check=True,
                    )
                    if h == 1:
                        emit_warm_mm()

            def emit_epilogue(b, P_ps):
                Psb = epip.tile([64, 512], F32, tag="Psb")
                nc.scalar.copy(Psb[0:4, :], P_ps[0:4, :])
                nc.vector.tensor_copy(Psb[32:36, :], P_ps[32:36, :])
                PT_ps = pswork.tile([128, NT * 4], F32, tag="work")
                for c in range(NT):
                    hb = 32 * (c // 4)
                    cc = c % 4
                    nc.tensor.transpose(
                        PT_ps[:, 4 * c : 4 * (c + 1)],
                        Psb[hb : hb + 4, 128 * cc : 128 * (cc + 1)],
                        identity32[hb : hb + 4, hb : hb + 4],
                    )
                PT = epip.tile([128, NT, 4], F32, tag="PT")
                nc.vector.tensor_copy(
                    PT[:, :, :], PT_ps[:, :].rearrange("p (t f) -> p t f", f=4)
                )
                tmp = epip.tile([128, NT, 3], F32, tag="tmp")
                a0, a1 = bass.broadcast_tensor_aps(pos16f[b][:, :, :], PT[:, :, 3:4])
                nc.gpsimd.tensor_mul(tmp[:, :, :], a0, a1)
                nc.gpsimd.tensor_sub(pre_all[:, b, :, :], tmp[:, :, :], PT[:, :, 0:3])

            # one P bank per batch; the two n-halves accumulate concurrently
            # at partitions 0-3 / 32-35 (PE column groups 0 / 1)
            P_all = {
                b: psP.tile([128, 512], F32, tag=f"P{b}", name=f"P{b}")
                for b in range(BPC)
            }
            P_ps = P_all

            # emission order tracks data arrival (engine queues are in-order):
            # b0 aug (pos only) -> b0 kv transposes -> b0 tiles 0..3 ->
            # b1 prologue (kv arrives later) -> interleaved remainder
            emit_aug(0)
            emit_kvT(0)
            for t in range(4):
                emit_tile(0, t, P_ps[0])
            emit_aug(1)
            emit_kvT(1)
            order = []
            for t in range(4):
                order += [(1, t), (0, t + 4)]
            order += [(1, t) for t in range(4, NT)]
            for b, t in order:
                emit_tile(b, t, P_ps[b])
            emit_epilogue(0, P_ps[0])
            emit_epilogue(1, P_ps[1])

            act = constp.tile([128, BPC, NT, 3], F32)
            nc.scalar.activation(act[:, :, :, :], pre_all[:, :, :, :], AF.Tanh)
            actf = constp.tile([128, BPC, NT, 3], F32)
            nc.gpsimd.tensor_scalar_mul(actf[:, :, :, :], act[:, :, :, :], ACTION_SCALE)
            nc.sync.dma_start(
                out=out_ext[:, :, :, :],
                in_=actf[:, :, :, :],
            )

    nc.compile()
    return nc


_NC_CACHE = {}


def _get_nc():
    if "nc" not in _NC_CACHE:
        _NC_CACHE["nc"] = build_nc()
    return _NC_CACHE["nc"]


def kernel(**inputs):
    kv = np.ascontiguousarray(np.asarray(inputs["kv"], dtype=np.float32))
    pos = np.ascontiguousarray(np.asarray(inputs["positions"], dtype=np.float32))
    assert kv.shape == (B, N, CKV) and pos.shape == (B, N, 3)
    nc = _get_nc()
    in_maps = [
        {
            "kv": kv[i * BPC : (i + 1) * BPC],
            "positions": pos[i * BPC : (i + 1) * BPC],
        }
        for i in range(NCORES)
    ]
    last_err = None
    for attempt in range(3):
        try:
            res = run_bass_kernel_spmd(nc, in_maps, core_ids=list(range(NCORES)))
            break
        except Exception as e:  # transient NRT device-state races between procs
            last_err = e
            if attempt == 2:
                raise
            time.sleep(2.0 * (attempt + 1))
    outs = res.results
    # out dump is [p, b, t, d] with row n = 8p + t -> [b, n, d]
    full = [
        np.ascontiguousarray(
            outs[i]["out"].transpose(1, 0, 2, 3).reshape(BPC, N, 3)
        )
        for i in range(NCORES)
    ]
    return np.concatenate(full, axis=0)


if __name__ == "__main__":
    rng = np.random.default_rng(0)
    kv = rng.standard_normal((B, N, CKV), dtype=np.float32)
    pos = rng.standard_normal((B, N, 3), dtype=np.float32)
    out = kernel(kv=kv, positions=pos)
    print("out", out.shape, out.dtype, float(np.abs(out).max()))
